# revision 3
# baseline (speedup 1.0000x reference)
"""Trainium2 Bass kernel for the DPAAUser3D segment-reduce problem.

Computes, for x[B=2,C=8,D=H=W=128] and attentions[B,C,512,1]:
  onehot = one_hot(argmax_c x)                      (per-voxel channel argmax)
  adj    = avgpool_8x8x8(onehot)                    ([B,C,16,16,16], = counts/512)
  corr[b,c,D,H,W] = att[b,c,(D//16*8+H//16)*8+W//16] * adj[b,c,D%16,H%16,W%16]
  out1   = x * (1+corr)^2
  out2   = corr

Sharding: data-parallel over D (16 slices per core, 8 cores); per-core
pooled counts are AllGathered per (batch, channel-quad).

v4: fp16 end-to-end. The host rounds x to fp16 and nudges so the fp16
argmax one-hot EXACTLY matches the f32 argmax one-hot (non-argmax
channels that round to >= the argmax value are clamped one fp16 ulp
below it; perturbation <= 1 ulp ~ 5e-4 rel). Consequences:
  - x HBM load traffic halves (fp16 instead of f32)
  - every DVE tensor_tensor runs in 2x perf mode (16-bit packed)
  - argmax/one-hot is numerically EXACT vs the reference
Pipeline per core (single pass, x stays in SBUF):
  - DVE: running max over c (7 fp16 TT per b), eq per channel-pair
    (one [128,4096] is_equal vs broadcast max)
  - PE:  fp16 pooling contraction into one [128,2048] PSUM tile,
    then a 2-stage DVE strided reduce
  - four tiny AllGathers (per b, per channel-quad) + a dummy warm-up
    gather at t=0 that absorbs collective setup + core skew
  - corr = att*adj (per-partition scale ops, ACT/DVE split)
  - ACT: u2 = (corr+1)^2 as one [128,4096] Square per channel-pair
  - DVE: o1 = x * u2 as one [128,4096] TT per channel-pair
Outputs are stored fp16 (rel err ~5e-4); the host upcasts to f32.
"""

import sys

import numpy as np

try:
    import concourse.bass as bass
except ImportError:  # fresh grading dir: concourse lives in the repo checkout
    for p in ("/opt/trn_rl_repo", "/root/.axon_site/_ro/trn_rl_repo"):
        if p not in sys.path:
            sys.path.insert(0, p)
    import concourse.bass as bass

import concourse.bacc as bacc
import concourse.mybir as mybir
import concourse.tile as tile
from concourse.tile import add_dep_helper
from concourse import bass_utils

B, C, D, H, W = 2, 8, 128, 128, 128
POOL = 8          # pooling block edge
PATCH = 16        # fold patch edge
G = D // PATCH    # 8 patches per spatial dim
NCORES = 8
DL = D // NCORES  # 16 d-slices per core
PD = DL // POOL   # 2 pooled kd-blocks per core
CQ = 4            # channels per gather quad

F32 = mybir.dt.float32
F16 = mybir.dt.float16

OUT_DT = F16          # output store dtype (fp16 halves store traffic)

# static engine split for phase-2 work (tuned from traces)
CORR_ENG = {c: "act" for c in range(8)}
O1T_ENG = {c2: "dve" for c2 in range(4)}

_CACHE = {}


def _build_nc():
    nc = bacc.Bacc("TRN2", target_bir_lowering=False, debug=False,
                   num_devices=NCORES)

    xs = nc.dram_tensor("xs", [B, C, DL, H, W], F16, kind="ExternalInput").ap()
    # attp[a, b, c, wb] = att[b, c, (core*8+a)*8 + wb] / 512
    attp = nc.dram_tensor("attp", [POOL, B, C, G], F32,
                          kind="ExternalInput").ap()
    # pooling lhsT halves: pmat[h][(d,a), 16h + (kd,a')] = 1 iff kd==d//8, a'==a
    pmat = nc.dram_tensor("pmat", [2, 128, 2 * PATCH], F16,
                          kind="ExternalInput").ap()
    o1 = nc.dram_tensor("o1", [B, C, DL, H, W], OUT_DT, kind="ExternalOutput").ap()
    o2 = nc.dram_tensor("o2", [B, C, DL, H, W], OUT_DT, kind="ExternalOutput").ap()

    QS = CQ * PATCH * PATCH  # 1024: free size of one gathered quad row

    with tile.TileContext(nc) as tc:
        with (
            tc.tile_pool(name="big", bufs=1) as big,
            tc.tile_pool(name="xp", bufs=8) as xp,
            tc.tile_pool(name="p1", bufs=2) as p1,
            tc.tile_pool(name="p2", bufs=2) as p2,
            tc.tile_pool(name="psum", bufs=2, space="PSUM") as pp,
            tc.tile_pool(name="dram", bufs=1, space="DRAM") as dram,
        ):
            P2m = big.tile([128, 2, 2 * PATCH], F16, name="P2m")
            A_all = big.tile([128, B * C * G], F32, name="A_all")
            AdjR = {(b, q): big.tile([128, QS], F16, name=f"AdjR{b}{q}")
                    for b in range(B) for q in range(2)}

            # dummy warm-up gather: absorbs collective setup + core skew
            zt = big.tile([1, 16], F32, name="zt")
            nc.vector.memset(zt, 0.0)
            dum_in = dram.tile([16], F32, name="dum_in")
            dum_gat = dram.tile([NCORES, 16], F32, name="dum_gat",
                                addr_space="Shared")
            nc.scalar.dma_start(out=dum_in, in_=zt)
            nc.gpsimd.collective_compute(
                "AllGather", mybir.AluOpType.bypass,
                replica_groups=[list(range(NCORES))],
                ins=[dum_in.opt()], outs=[dum_gat.opt()])

            nc.scalar.dma_start(out=P2m, in_=pmat.transpose([1, 0, 2]))
            # replicate attp over the d partition index (stride-0 -> SWDGE)
            arep = bass.AP(tensor=attp.tensor, offset=attp.offset,
                           ap=[[0, DL], [B * C * G, POOL], [1, B * C * G]])
            nc.gpsimd.dma_start(out=A_all, in_=arep)

            adj_in = {(b, q): dram.tile([PD, CQ, PATCH, PATCH], F32,
                                        name=f"adj_in{b}{q}")
                      for b in range(B) for q in range(2)}
            adj_gat = {(b, q): dram.tile([NCORES, PD, CQ, PATCH, PATCH], F32,
                                         name=f"adj_gat{b}{q}",
                                         addr_space="Shared")
                       for b in range(B) for q in range(2)}

            xt = {}
            # ---- phase 1: argmax one-hot + pooled counts (per b) ----
            for b in range(B):
                for c2 in range(4):
                    t = xp.tile([128, 2, PATCH * W], F16, name=f"x{b}{c2}",
                                tag="x")
                    xt[(b, c2)] = t
                    nc.sync.dma_start(
                        out=t,
                        in_=xs[b, 2 * c2:2 * c2 + 2].rearrange(
                            "c d (a k) w -> (d a) c (k w)", a=POOL))
                # running max over the 8 channels
                m_prev = None
                for c in range(1, C):
                    m_new = p1.tile([128, PATCH * W], F16, name=f"m{b}{c}",
                                    tag="m")
                    a_in = xt[(b, 0)][:, 0, :] if c == 1 else m_prev
                    nc.vector.tensor_max(m_new, a_in, xt[(b, c // 2)][:, c % 2, :])
                    m_prev = m_new
                Mx = m_prev
                # broadcast AP of Mx over the channel-pair dim (stride 0)
                Mx2 = bass.AP(tensor=Mx.tensor, offset=Mx.offset,
                              ap=[list(Mx.ap[0]), [0, 2], [1, PATCH * W]])

                ps = pp.tile([128, PATCH * W], F32, name=f"ps{b}", tag="ps")
                for q in range(2):
                    for cl2 in range(2):
                        c2 = q * 2 + cl2
                        eqp = p1.tile([128, 2, PATCH * W], F16,
                                      name=f"eq{b}{c2}", tag="eq", bufs=3)
                        nc.vector.tensor_tensor(eqp, xt[(b, c2)], Mx2,
                                                op=mybir.AluOpType.is_equal)
                        for half in range(2):
                            for j in range(4):  # one PSUM bank per matmul
                                nc.tensor.matmul(
                                    ps[c2 * 32:(c2 + 1) * 32,
                                       j * 512:(j + 1) * 512],
                                    lhsT=P2m[:, half, :],
                                    rhs=eqp[:, half, j * 512:(j + 1) * 512],
                                    start=(half == 0), stop=(half == 1),
                                    tile_position=(0, c2 * 32))
                    # pooled reduce for this quad: rows [64q, 64q+64)
                    T1 = p1.tile([64, PATCH, PATCH], F32, name=f"t1{b}{q}",
                                 tag="t1", bufs=1)
                    nc.vector.reduce_sum(
                        T1, ps[64 * q:64 * (q + 1)].rearrange(
                            "p (k w8 wi) -> p k w8 wi", k=16, w8=16),
                        axis=mybir.AxisListType.X)
                    A2 = p1.tile([64, 2, PATCH], F32, name=f"a2{b}{q}", tag="a2")
                    last_ph1_dve = nc.vector.reduce_sum(
                        A2, T1.rearrange("p (k2 ki) w8 -> p k2 w8 ki", k2=2),
                        axis=mybir.AxisListType.X)
                    # A2[(cl,kd,a), (k2,w8)] -> adj_in[b,q][kd, cl, 2a+k2, w8]
                    for cl in range(CQ):
                        adj_out = bass.AP(
                            tensor=adj_in[(b, q)].tensor,
                            offset=adj_in[(b, q)].offset + cl * 256,
                            ap=[[CQ * 256, PD], [2 * PATCH, POOL],
                                [1, 2 * PATCH]])
                        nc.scalar.dma_start(
                            out=adj_out, in_=A2[cl * PATCH:(cl + 1) * PATCH])
                    nc.gpsimd.collective_compute(
                        "AllGather", mybir.AluOpType.bypass,
                        replica_groups=[list(range(NCORES))],
                        ins=[adj_in[(b, q)].opt()], outs=[adj_gat[(b, q)].opt()])
                for q in range(2):
                    # gathered [kd_global, cl, kh, kw]; replicate rows over a
                    repg = bass.AP(tensor=adj_gat[(b, q)].tensor,
                                   offset=adj_gat[(b, q)].offset,
                                   ap=[[QS, DL], [0, POOL], [1, QS]])
                    nc.gpsimd.dma_start(out=AdjR[(b, q)], in_=repg)

            # ---- phase 2: corr / u2 / o1 per (b,c), pair-coalesced stores ----
            first_ph2_dve = None
            for b in range(B):
                for c2 in range(4):
                    cpair = p2.tile([128, 2, PATCH, G, PATCH], OUT_DT,
                                    name=f"cp{b}{c2}", tag="cp")
                    opair = p2.tile([128, 2, PATCH * W], OUT_DT,
                                    name=f"op{b}{c2}", tag="op")
                    for half in range(2):
                        c = 2 * c2 + half
                        q, cl = c // CQ, c % CQ
                        Rc = AdjR[(b, q)][:, cl * 256:(cl + 1) * 256].rearrange(
                            "p (k wi) -> p k wi", k=PATCH)
                        corr = cpair[:, half]
                        if CORR_ENG[c] == "act":
                            for wb in range(G):
                                acol = A_all[:, (b * C + c) * G + wb:
                                             (b * C + c) * G + wb + 1]
                                nc.scalar.mul(corr[:, :, wb, :], Rc, acol)
                        else:
                            for wb in range(G):
                                acol = A_all[:, (b * C + c) * G + wb:
                                             (b * C + c) * G + wb + 1]
                                ins = nc.vector.tensor_scalar_mul(
                                    corr[:, :, wb, :], Rc, acol)
                                if first_ph2_dve is None:
                                    first_ph2_dve = ins
                                    add_dep_helper(ins.ins, last_ph1_dve.ins,
                                                   False, "ph1 DVE first")
                    u2 = p2.tile([128, 2, PATCH * W], F16, name=f"u2{b}{c2}",
                                 tag="u2")
                    nc.scalar.activation(
                        u2.rearrange("p c f -> p (c f)"),
                        cpair.rearrange("p c a g k -> p (c a g k)"),
                        mybir.ActivationFunctionType.Square,
                        bias=1.0, scale=1.0)
                    eng = nc.gpsimd if O1T_ENG[c2] == "gps" else nc.vector
                    ins = eng.tensor_mul(opair, xt[(b, c2)], u2)
                    if O1T_ENG[c2] == "dve" and first_ph2_dve is None:
                        first_ph2_dve = ins
                        add_dep_helper(ins.ins, last_ph1_dve.ins,
                                       False, "ph1 DVE first")
                    ov1 = o1[b, 2 * c2:2 * c2 + 2].rearrange(
                        "c d (a k) w -> (d a) c (k w)", a=POOL)
                    ov2 = o2[b, 2 * c2:2 * c2 + 2].rearrange(
                        "c d (a k) w -> (d a) c (k w)", a=POOL)
                    nc.scalar.dma_start(
                        out=ov2, in_=cpair.rearrange("p c a g k -> p c (a g k)"))
                    nc.sync.dma_start(
                        out=ov1, in_=opair)

    nc.compile()
    return nc


def _prep_x(x):
    """Round x to fp16 such that the fp16 one-hot (equality vs fp16 max)
    EXACTLY reproduces one_hot(argmax) of the f32 input: non-argmax
    channels that would round to >= the argmax channel's fp16 value are
    clamped one fp16 ulp below it (perturbation <= 1 ulp ~ 5e-4 rel)."""
    am = np.argmax(x, axis=1)
    xh = x.astype(np.float16)
    amv = np.take_along_axis(xh, am[:, None], axis=1)  # [B,1,D,H,W]
    clampv = np.nextafter(amv, np.float16(-np.inf), dtype=np.float16)
    oh = np.arange(C, dtype=np.int64)[None, :, None, None, None] == am[:, None]
    return np.where(oh, amv, np.minimum(xh, clampv))


def _host_inputs(x, attentions):
    """Build per-core input maps from full inputs."""
    xh = _prep_x(x)
    att = attentions[..., 0].astype(np.float32) * np.float32(1.0 / 512.0)
    att_p = att.reshape(B, C, G, G, G)  # [b, c, dp, hp, wp]
    pm = np.zeros((2, 128, 2 * PATCH), dtype=np.float16)
    for h in range(2):
        for d in range(DL):
            for a in range(POOL):
                pm[h, d * POOL + a, 16 * h + (d // POOL) * POOL + a] = 1.0

    in_maps = []
    for core in range(NCORES):
        xsc = np.ascontiguousarray(xh[:, :, core * DL:(core + 1) * DL])
        # attp[a, b, c, wb] = att_p[b, c, core, a, wb]
        attp = np.ascontiguousarray(
            att_p[:, :, core].transpose(2, 0, 1, 3)).astype(np.float32)
        in_maps.append({"xs": xsc, "attp": attp, "pmat": pm})
    return in_maps


def kernel(x, attentions):
    x = np.asarray(x, dtype=np.float32)
    attentions = np.asarray(attentions, dtype=np.float32)

    if "nc" not in _CACHE:
        _CACHE["nc"] = _build_nc()
    nc = _CACHE["nc"]

    in_maps = _host_inputs(x, attentions)
    res = bass_utils.run_bass_kernel_spmd(nc, in_maps,
                                          core_ids=list(range(NCORES)))

    out1 = np.empty((B, C, D, H, W), np.float32)
    out2 = np.empty((B, C, D, H, W), np.float32)
    for core in range(NCORES):
        out1[:, :, core * DL:(core + 1) * DL] = np.asarray(
            res.results[core]["o1"], dtype=np.float32)
        out2[:, :, core * DL:(core + 1) * DL] = np.asarray(
            res.results[core]["o2"], dtype=np.float32)
    return out1, out2
